# revision 4
# baseline (speedup 1.0000x reference)
"""Trainium2 Bass kernel for single-head causal attention (nn_Head).

Reference computation (fp32):
    q = x @ Wq; k = x @ Wk; v = x @ Wv        # x [B,T,C]=[256,256,768], W [768,64]
    S = (q @ k^T) / 8, causal-masked, softmax over s
    out = S @ v                                # [256,256,64]

Strategy:
  - Data-parallel over batch B across 8 NeuronCores (32 batches/core),
    projection weights replicated.
  - Host-side layout prep: x is pre-transposed to [B, C, T] and cast to
    bf16 so the device consumes xT tiles [c,t] directly (matmul contracts
    the partition dim; both operands need C on partitions). Wq|Wk are
    concatenated to one [768,128] stacked projection.
  - On-chip per batch: qkT = (Wq|Wk)^T xT (one M=128 matmul chain),
    v = xT^T Wv (natural layout), S^T blocks = k^T q (only the 3
    causally-live 128x128 blocks), exp via ACT (no max-subtraction:
    |S|/8 <= ~2.5 so exp is safe in fp32/bf16), causal mask applied as a
    multiplicative upper-triangular bf16 mask on the two diagonal blocks,
    and out = P v with a ones-column appended to v so the softmax
    denominator falls out of the same matmul. Final normalize on DVE.
"""

import sys
import os

for _p in ("/opt/trn_rl_repo", os.path.dirname(os.path.abspath(__file__))):
    if _p not in sys.path:
        sys.path.insert(0, _p)

import numpy as np
import ml_dtypes

import concourse.bass as bass
import concourse.mybir as mybir
import concourse.tile as tile
from concourse.bass_utils import run_bass_kernel_spmd

BF16 = ml_dtypes.bfloat16
F32 = mybir.dt.float32
BF = mybir.dt.bfloat16

B, T, C, H = 256, 256, 768, 64
NCORES = 8
BS = B // NCORES          # batches per core
NC_CHUNKS = C // 128      # 6 contraction chunks
SCALE = 1.0 / np.sqrt(H)  # 0.125

# ---------------------------------------------------------------------------
# Walrus on this container rejects instructions carrying more than one sync
# wait; the Tile tail drain aggregates one wait per outstanding semaphore.
# Spread them across preceding NOPs on the same (SP) engine queue.
# ---------------------------------------------------------------------------


def _split_sync_waits(nc, limit=1):
    """Move excess per-instruction sem waits onto same-engine NOPs inserted
    immediately before the instruction (engine queue order preserved)."""
    n_split = 0
    for f in nc.m.functions:
        for bb in f.blocks:
            il = bb.instructions
            if not any(
                ins.sync_info is not None
                and ins.sync_info.on_wait
                and len(ins.sync_info.on_wait) > limit
                for ins in il
            ):
                continue
            new_list = []
            for ins in il:
                si = ins.sync_info
                waits = list(si.on_wait) if si is not None and si.on_wait else []
                if len(waits) > limit:
                    keep = waits[len(waits) - limit :]
                    spill = waits[: len(waits) - limit]
                    for w in spill:
                        nop = mybir.InstNoOp(
                            name=nc.get_next_instruction_name(),
                            engine=ins.engine,
                            ins=[],
                            outs=[],
                            sync_info=mybir.SyncInfo(on_wait=[w], on_update=[]),
                            bass_nofuse=True,
                        )
                        nc.register_instruction(nop)
                        new_list.append(nop)
                        n_split += 1
                    si.on_wait = keep
                new_list.append(ins)
            il[:] = new_list
    return n_split


def build_program():
    nc = bass.Bass()

    xt_d = nc.dram_tensor("xt", [BS, C, T], BF, kind="ExternalInput")
    wqk_d = nc.dram_tensor("wqk", [C, 128], BF, kind="ExternalInput")
    wv_d = nc.dram_tensor("wv", [C, H], BF, kind="ExternalInput")
    um_d = nc.dram_tensor("umask", [128, 128], BF, kind="ExternalInput")
    out_d = nc.dram_tensor("out", [BS, T, H], F32, kind="ExternalOutput")

    with tile.TileContext(nc) as tc:
        with (
            tc.tile_pool(name="consts", bufs=1) as consts,
            tc.tile_pool(name="xp", bufs=3) as xp,
            tc.tile_pool(name="qk", bufs=3) as qkp,
            tc.tile_pool(name="vp", bufs=4) as vp,
            tc.tile_pool(name="ptp", bufs=3) as ptp,
            tc.tile_pool(name="rp", bufs=4) as rp,
            tc.tile_pool(name="op", bufs=2) as op,
            tc.tile_pool(name="ps_qk", bufs=2, space="PSUM") as ps_qk,
            tc.tile_pool(name="ps_st", bufs=2, space="PSUM") as ps_st,
            tc.tile_pool(name="ps_v", bufs=2, space="PSUM") as ps_v,
            tc.tile_pool(name="ps_av", bufs=2, space="PSUM") as ps_av,
        ):
            wqk = consts.tile([128, NC_CHUNKS, 128], BF)
            nc.sync.dma_start(wqk[:], wqk_d.rearrange("(n p) m -> p n m", p=128))
            wv = consts.tile([128, NC_CHUNKS, H], BF)
            nc.sync.dma_start(wv[:], wv_d.rearrange("(n p) m -> p n m", p=128))
            um = consts.tile([128, 128], BF)
            nc.sync.dma_start(um[:], um_d[:])

            ostage = None
            for b in range(BS):
                # ---- load xT[b] : [c, t] as 6 chunks of [128, 256] -------
                xt = xp.tile([128, NC_CHUNKS, T], BF, tag="xt")
                nc.sync.dma_start(xt[:], xt_d[b].rearrange("(n p) m -> p n m", p=128))

                # ---- stacked QK projection: qkT = (Wq|Wk)^T @ x^T --------
                qk_ps = ps_qk.tile([128, T], F32, tag="qk")
                for ci in range(NC_CHUNKS):
                    nc.tensor.matmul(
                        qk_ps[:],
                        wqk[:, ci, :],
                        xt[:, ci, :],
                        start=(ci == 0),
                        stop=(ci == NC_CHUNKS - 1),
                    )
                # split into qT (psum parts 0:64) / kT (parts 64:128), both
                # landing on SBUF partitions 0:64 (DVE reads follow the src AP)
                qt = qkp.tile([64, T], BF, tag="qt")
                kt = qkp.tile([64, T], BF, tag="kt")
                nc.vector.tensor_copy(qt[:], qk_ps[0:64, :])
                nc.vector.tensor_copy(kt[:], qk_ps[64:128, :])

                # ---- V projection (natural [s, h]) + ones column ---------
                vone = []
                for ti in range(2):
                    v_ps = ps_v.tile([128, H], F32, tag="v")
                    for ci in range(NC_CHUNKS):
                        nc.tensor.matmul(
                            v_ps[:],
                            xt[:, ci, ti * 128 : (ti + 1) * 128],
                            wv[:, ci, :],
                            start=(ci == 0),
                            stop=(ci == NC_CHUNKS - 1),
                        )
                    vo = vp.tile([128, H + 1], BF, tag="vone")
                    nc.vector.tensor_copy(vo[:, 0:H], v_ps[:])
                    nc.gpsimd.memset(vo[:, H : H + 1], 1.0)
                    vone.append(vo)

                # ---- S^T blocks: st[s, t] = sum_h kT[h,s] qT[h,t] --------
                # block layout in one psum tile [128, 384]:
                #   [:, 0:256]   = s-chunk 0 x (t0, t1)
                #   [:, 256:384] = s-chunk 1 x t1
                st_ps = ps_st.tile([128, 384], F32, tag="st")
                nc.tensor.matmul(
                    st_ps[:, 0:T], kt[:, 0:128], qt[:], start=True, stop=True
                )
                nc.tensor.matmul(
                    st_ps[:, T : T + 128],
                    kt[:, 128:256],
                    qt[:, 128:256],
                    start=True,
                    stop=True,
                )

                # ---- exp (with 1/8 scale) -> P^T in bf16 -----------------
                pt = ptp.tile([128, 384], BF, tag="pt")
                nc.scalar.activation(
                    pt[:, 0:T], st_ps[:, 0:T],
                    mybir.ActivationFunctionType.Exp, scale=SCALE,
                )
                nc.scalar.activation(
                    pt[:, T : T + 128], st_ps[:, T : T + 128],
                    mybir.ActivationFunctionType.Exp, scale=SCALE,
                )
                # causal mask on the two diagonal blocks (multiplicative)
                nc.vector.tensor_mul(pt[:, 0:128], pt[:, 0:128], um[:])
                nc.vector.tensor_mul(pt[:, T : T + 128], pt[:, T : T + 128], um[:])

                # ---- out = P @ [v | 1] : denominators ride along ---------
                if b % 4 == 0:
                    ostage = op.tile([128, 8, H], F32, tag="o")
                slot = (b % 4) * 2

                av0 = ps_av.tile([128, H + 1], F32, tag="av")
                nc.tensor.matmul(av0[:], pt[:, 0:128], vone[0][:], start=True, stop=True)
                r0 = rp.tile([128, 1], F32, tag="r")
                nc.vector.reciprocal(r0[:], av0[:, H : H + 1])
                nc.vector.tensor_scalar_mul(ostage[:, slot, :], av0[:, 0:H], r0[:])

                av1 = ps_av.tile([128, H + 1], F32, tag="av")
                nc.tensor.matmul(
                    av1[:], pt[:, 128:256], vone[0][:], start=True, stop=False
                )
                nc.tensor.matmul(
                    av1[:], pt[:, T : T + 128], vone[1][:], start=False, stop=True
                )
                r1 = rp.tile([128, 1], F32, tag="r")
                nc.vector.reciprocal(r1[:], av1[:, H : H + 1])
                nc.vector.tensor_scalar_mul(ostage[:, slot + 1, :], av1[:, 0:H], r1[:])

                # ---- store 4 batches at a time ---------------------------
                if b % 4 == 3:
                    g = b // 4
                    dst = out_d[g * 4 : (g + 1) * 4].rearrange(
                        "b (c p) h -> p (b c) h", p=128
                    )
                    nc.sync.dma_start(dst, ostage[:])

    _split_sync_waits(nc, limit=1)
    nc.finalize()
    return nc


_NC = None


def _get_nc():
    global _NC
    if _NC is None:
        _NC = build_program()
    return _NC


def _prep_inputs(x, Wq, Wk, Wv):
    xt = np.ascontiguousarray(np.asarray(x, dtype=np.float32).transpose(0, 2, 1))
    xt = xt.astype(BF16)  # [B, C, T]
    wqk = np.concatenate(
        [np.asarray(Wq, np.float32), np.asarray(Wk, np.float32)], axis=1
    ).astype(BF16)
    wv = np.asarray(Wv, np.float32).astype(BF16)
    um = np.triu(np.ones((128, 128), np.float32)).astype(BF16)  # keep t >= s
    in_maps = []
    for i in range(NCORES):
        in_maps.append(
            {
                "xt": np.ascontiguousarray(xt[i * BS : (i + 1) * BS]),
                "wqk": wqk,
                "wv": wv,
                "umask": um,
            }
        )
    return in_maps


def _run(x, Wq, Wk, Wv, trace=False):
    nc = _get_nc()
    in_maps = _prep_inputs(x, Wq, Wk, Wv)
    res = run_bass_kernel_spmd(nc, in_maps, list(range(NCORES)), trace=trace)
    out = np.concatenate([res.results[i]["out"] for i in range(NCORES)], axis=0)
    return np.ascontiguousarray(out.astype(np.float32)), res


def kernel(x, Wq, Wk, Wv):
    out, _ = _run(x, Wq, Wk, Wv, trace=False)
    return out


# revision 8
# speedup vs baseline: 1.0732x; 1.0732x over previous
"""Trainium2 Bass kernel for single-head causal attention (nn_Head).

Reference computation (fp32):
    q = x @ Wq; k = x @ Wk; v = x @ Wv        # x [B,T,C]=[256,256,768], W [768,64]
    S = (q @ k^T) / 8, causal-masked, softmax over s
    out = S @ v                                # [256,256,64]

Strategy:
  - Data-parallel over batch B across 8 NeuronCores (32 batches/core),
    projection weights replicated.
  - Host-side layout prep: x is pre-transposed to [B, C, T] and cast to
    bf16 so the device consumes xT tiles [c,t] directly (matmul contracts
    the partition dim; both operands need C on partitions). Wq|Wk are
    concatenated to one [768,128] stacked projection.
  - On-chip per batch: qkT = (Wq|Wk)^T xT (one M=128 matmul chain),
    v = xT^T Wv (natural layout), S^T blocks = k^T q (only the 3
    causally-live 128x128 blocks), exp via ACT (no max-subtraction:
    |S|/8 <= ~2.5 so exp is safe in fp32/bf16), causal mask applied as a
    multiplicative upper-triangular bf16 mask on the two diagonal blocks,
    and out = P v with a ones-column appended to v so the softmax
    denominator falls out of the same matmul. Final normalize on DVE.
"""

import sys
import os

for _p in ("/opt/trn_rl_repo", os.path.dirname(os.path.abspath(__file__))):
    if _p not in sys.path:
        sys.path.insert(0, _p)

import numpy as np
import ml_dtypes

import concourse.bass as bass
import concourse.mybir as mybir
import concourse.tile as tile
from concourse.bass_utils import run_bass_kernel_spmd

BF16 = ml_dtypes.bfloat16
F32 = mybir.dt.float32
BF = mybir.dt.bfloat16

B, T, C, H = 256, 256, 768, 64
NCORES = 8
BS = B // NCORES          # batches per core
NC_CHUNKS = C // 128      # 6 contraction chunks
SCALE = 1.0 / np.sqrt(H)  # 0.125

# ---------------------------------------------------------------------------
# Walrus on this container rejects instructions carrying more than one sync
# wait; the Tile tail drain aggregates one wait per outstanding semaphore.
# Spread them across preceding NOPs on the same (SP) engine queue.
# ---------------------------------------------------------------------------


def _split_sync_waits(nc, limit=1):
    """Move excess per-instruction sem waits onto same-engine NOPs inserted
    immediately before the instruction (engine queue order preserved)."""
    n_split = 0
    for f in nc.m.functions:
        for bb in f.blocks:
            il = bb.instructions
            if not any(
                ins.sync_info is not None
                and ins.sync_info.on_wait
                and len(ins.sync_info.on_wait) > limit
                for ins in il
            ):
                continue
            new_list = []
            for ins in il:
                si = ins.sync_info
                waits = list(si.on_wait) if si is not None and si.on_wait else []
                if len(waits) > limit:
                    keep = waits[len(waits) - limit :]
                    spill = waits[: len(waits) - limit]
                    for w in spill:
                        nop = mybir.InstNoOp(
                            name=nc.get_next_instruction_name(),
                            engine=ins.engine,
                            ins=[],
                            outs=[],
                            sync_info=mybir.SyncInfo(on_wait=[w], on_update=[]),
                            bass_nofuse=True,
                        )
                        nc.register_instruction(nop)
                        new_list.append(nop)
                        n_split += 1
                    si.on_wait = keep
                new_list.append(ins)
            il[:] = new_list
    return n_split


def build_program():
    nc = bass.Bass()

    xt_d = nc.dram_tensor("xt", [BS, C, T], BF, kind="ExternalInput")
    wqk_d = nc.dram_tensor("wqk", [C, 128], BF, kind="ExternalInput")
    wv_d = nc.dram_tensor("wv", [C, H], BF, kind="ExternalInput")
    um_d = nc.dram_tensor("umask2", [128, 256], BF, kind="ExternalInput")
    out_d = nc.dram_tensor("out", [BS, T, H], F32, kind="ExternalOutput")

    with tile.TileContext(nc) as tc:
        with (
            tc.tile_pool(name="consts", bufs=1) as consts,
            tc.tile_pool(name="xp", bufs=3) as xp,
            tc.tile_pool(name="qk", bufs=3) as qkp,
            tc.tile_pool(name="vp", bufs=4) as vp,
            tc.tile_pool(name="ptp", bufs=3) as ptp,
            tc.tile_pool(name="rp", bufs=4) as rp,
            tc.tile_pool(name="op", bufs=2) as op,
            tc.tile_pool(name="ps_qk", bufs=2, space="PSUM") as ps_qk,
            tc.tile_pool(name="ps_st", bufs=2, space="PSUM") as ps_st,
            tc.tile_pool(name="ps_v", bufs=2, space="PSUM") as ps_v,
            tc.tile_pool(name="ps_av", bufs=2, space="PSUM") as ps_av,
        ):
            wqk = consts.tile([128, NC_CHUNKS, 128], BF)
            nc.sync.dma_start(wqk[:], wqk_d.rearrange("(n p) m -> p n m", p=128))
            wv = consts.tile([128, NC_CHUNKS, H], BF)
            nc.sync.dma_start(wv[:], wv_d.rearrange("(n p) m -> p n m", p=128))
            um2 = consts.tile([128, 256], BF)
            nc.sync.dma_start(um2[:], um_d[:])

            ostage = None
            for g2 in range(BS // 2):
                # ---- load xT for a PAIR of batches: [c, (b, t)] ----------
                xt = xp.tile([128, NC_CHUNKS, 2, T], BF, tag="xt")
                for bi in range(2):
                    nc.sync.dma_start(
                        xt[:, :, bi, :],
                        xt_d[2 * g2 + bi].rearrange("(n p) m -> p n m", p=128),
                    )

                # ---- stacked QK projection for both batches (N=512) ------
                qk_ps = ps_qk.tile([128, 2 * T], F32, tag="qk")
                for ci in range(NC_CHUNKS):
                    nc.tensor.matmul(
                        qk_ps[:],
                        wqk[:, ci, :],
                        xt[:, ci, :, :],
                        start=(ci == 0),
                        stop=(ci == NC_CHUNKS - 1),
                    )
                # one full-width bf16 copy; kT additionally shifted down to
                # partitions 0:64 (DVE reads follow the src AP)
                qk_sb = qkp.tile([128, 2 * T], BF, tag="qksb")
                nc.vector.tensor_copy(qk_sb[:], qk_ps[:])
                kt = qkp.tile([64, 2 * T], BF, tag="kt")
                nc.vector.tensor_copy(kt[:], qk_sb[64:128, :])

                for bi in range(2):
                    b = 2 * g2 + bi
                    boff = bi * T
                    qt_b = qk_sb[0:64, boff : boff + T]

                    # ---- V projection (natural [s, h]) + ones column -----
                    vone = []
                    for ti in range(2):
                        v_ps = ps_v.tile([128, H], F32, tag="v")
                        for ci in range(NC_CHUNKS):
                            nc.tensor.matmul(
                                v_ps[:],
                                xt[:, ci, bi, ti * 128 : (ti + 1) * 128],
                                wv[:, ci, :],
                                start=(ci == 0),
                                stop=(ci == NC_CHUNKS - 1),
                            )
                        vo = vp.tile([128, H + 1], BF, tag="vone")
                        nc.vector.tensor_copy(vo[:, 0:H], v_ps[:])
                        nc.gpsimd.memset(vo[:, H : H + 1], 1.0)
                        vone.append(vo)

                    # ---- S^T blocks: st[s, t] = sum_h kT[h,s] qT[h,t] ----
                    # layout in one psum tile [128, 384]:
                    #   [:, 0:128]   = s1 x t1   (diagonal block)
                    #   [:, 128:256] = s0 x t0   (diagonal block)
                    #   [:, 256:384] = s0 x t1   (full block)
                    st_ps = ps_st.tile([128, 384], F32, tag="st")
                    nc.tensor.matmul(
                        st_ps[:, 0:128],
                        kt[:, boff + 128 : boff + 256],
                        qt_b[:, 128:256],
                        start=True,
                        stop=True,
                    )
                    nc.tensor.matmul(
                        st_ps[:, 128:384],
                        kt[:, boff : boff + 128],
                        qt_b[:],
                        start=True,
                        stop=True,
                    )

                    # ---- exp (with 1/8 scale) -> P^T bf16, one ACT op ----
                    pt = ptp.tile([128, 384], BF, tag="pt")
                    nc.scalar.activation(
                        pt[:], st_ps[:],
                        mybir.ActivationFunctionType.Exp, scale=SCALE,
                    )
                    # causal mask: both diagonal blocks adjacent -> one mul
                    nc.vector.tensor_mul(pt[:, 0:256], pt[:, 0:256], um2[:])

                    # ---- out = P @ [v | 1] -------------------------------
                    if b % 4 == 0:
                        ostage = op.tile([128, 8, H], F32, tag="o")
                    slot = (b % 4) * 2

                    av = ps_av.tile([128, 2, H + 1], F32, tag="av")
                    nc.tensor.matmul(
                        av[:, 0, :], pt[:, 128:256], vone[0][:], start=True, stop=True
                    )
                    nc.tensor.matmul(
                        av[:, 1, :], pt[:, 256:384], vone[0][:], start=True, stop=False
                    )
                    nc.tensor.matmul(
                        av[:, 1, :], pt[:, 0:128], vone[1][:], start=False, stop=True
                    )
                    rec = rp.tile([128, 2], F32, tag="r")
                    nc.vector.reciprocal(rec[:], av[:, :, H : H + 1])
                    nc.vector.tensor_scalar_mul(
                        ostage[:, slot, :], av[:, 0, 0:H], rec[:, 0:1]
                    )
                    nc.vector.tensor_scalar_mul(
                        ostage[:, slot + 1, :], av[:, 1, 0:H], rec[:, 1:2]
                    )

                    # ---- store 4 batches at a time -----------------------
                    if b % 4 == 3:
                        g = b // 4
                        dst = out_d[g * 4 : (g + 1) * 4].rearrange(
                            "b (c p) h -> p (b c) h", p=128
                        )
                        nc.sync.dma_start(dst, ostage[:])

    _split_sync_waits(nc, limit=1)
    nc.finalize()
    return nc


_NC = None


def _get_nc():
    global _NC
    if _NC is None:
        _NC = build_program()
    return _NC


def _prep_inputs(x, Wq, Wk, Wv):
    xt = np.ascontiguousarray(np.asarray(x, dtype=np.float32).transpose(0, 2, 1))
    xt = xt.astype(BF16)  # [B, C, T]
    wqk = np.concatenate(
        [np.asarray(Wq, np.float32), np.asarray(Wk, np.float32)], axis=1
    ).astype(BF16)
    wv = np.asarray(Wv, np.float32).astype(BF16)
    um = np.triu(np.ones((128, 128), np.float32)).astype(BF16)  # keep t >= s
    um2 = np.concatenate([um, um], axis=1)  # [128, 256] for both diag blocks
    in_maps = []
    for i in range(NCORES):
        in_maps.append(
            {
                "xt": np.ascontiguousarray(xt[i * BS : (i + 1) * BS]),
                "wqk": wqk,
                "wv": wv,
                "umask2": um2,
            }
        )
    return in_maps


def _run(x, Wq, Wk, Wv, trace=False):
    nc = _get_nc()
    in_maps = _prep_inputs(x, Wq, Wk, Wv)
    res = run_bass_kernel_spmd(nc, in_maps, list(range(NCORES)), trace=trace)
    out = np.concatenate([res.results[i]["out"] for i in range(NCORES)], axis=0)
    return np.ascontiguousarray(out.astype(np.float32)), res


def kernel(x, Wq, Wk, Wv):
    out, _ = _run(x, Wq, Wk, Wv, trace=False)
    return out
